# revision 2
# baseline (speedup 1.0000x reference)
"""Trainium2 Bass kernel for a 2-layer GCN (nn_MiniGNN).

Math (per GCNConv, symmetric norm, self loops):
    deg[d] = indeg[d] + 1;  dinv = deg^-1/2
    out[d] = dinv[d] * ( sum_{e: dst=d} dinv[src_e] * h[src_e]  +  dinv[d]*h[d] ) + b
with h = x @ W.  Layer 1 applies relu;  layer 2 output = (S a1) @ W2 + b2,
computed as S @ (a1 @ W2) (associativity) so both layers aggregate via the
same machinery.

Implementation: 8-core SPMD.  Nodes (and their incoming edges) are sharded
by destination across cores.  Per core, edges are sorted by (dst-block of
128, src-bucket of 25088) and padded to 128-edge chunks.  Each chunk is
gathered (dma_gather, 1024 rows / instruction over 4 SWDGE queues) and
scatter-added into a PSUM accumulator via a one-hot matrix built on DVE
(is_equal against an iota row) and a PE matmul.  dinv factorizes into the
gathered table rows (src side) and the PSUM eviction scale (dst side);
self loops are identity-matmul'd from the core-local table slice; biases
are fused into eviction.  Between layers, table slices are exchanged with
an AllGather collective.
"""
import sys

sys.path.insert(0, "/opt/trn_rl_repo")

import numpy as np
import ml_dtypes
from dataclasses import dataclass

from concourse import bass, bacc, mybir
import concourse.tile as tile
from concourse.bass_utils import run_bass_kernel_spmd
from concourse.library_config import mlp

bf16 = mybir.dt.bfloat16
f32 = mybir.dt.float32
i16 = mybir.dt.int16
np_bf16 = ml_dtypes.bfloat16

NCORES = 8
BLOCK = 128          # dst nodes per psum block
CHUNK = 128          # edges per matmul chunk
GCH = 8              # chunks per dma_gather instruction
GIDX = CHUNK * GCH   # 1024 indices per gather


@dataclass
class Cfg:
    n: int           # real node count
    din: int
    dh: int
    dout: int
    bucket_rows: int  # src rows per int16 bucket (<= 32768)

    @property
    def nbuckets(self):
        return 4

    @property
    def npad(self):
        # multiple of NCORES*BLOCK and of nbuckets*... bucket_rows covers npad
        per = -(-self.n // (NCORES * BLOCK)) * BLOCK
        return per * NCORES

    @property
    def per_core(self):
        return self.npad // NCORES

    @property
    def nblk(self):
        return self.per_core // BLOCK


FULL_CFG = Cfg(n=100000, din=256, dh=64, dout=128, bucket_rows=25088)


# ----------------------------------------------------------------- host prep

def host_prep(cfg: Cfg, edge_index: np.ndarray):
    """Bucket/sort/pad edges per core. Returns (chunks_bk, per-core streams, dinv)."""
    src = np.asarray(edge_index[0], dtype=np.int64)
    dst = np.asarray(edge_index[1], dtype=np.int64)
    n, npad, per_core, nblk, nb = cfg.n, cfg.npad, cfg.per_core, cfg.nblk, cfg.nbuckets

    deg = np.bincount(dst, minlength=n).astype(np.float64) + 1.0
    dinv = (1.0 / np.sqrt(deg)).astype(np.float32)
    dinv = np.concatenate([dinv, np.ones(npad - n, np.float32)])

    core = dst // per_core
    ngroups = nblk * nb
    per_core_data = []
    cnt_all = np.zeros((NCORES, ngroups), np.int64)
    sorted_per_core = []
    for m in range(NCORES):
        sel = core == m
        s = src[sel]
        dl = dst[sel] - m * per_core
        blk = dl // BLOCK
        dloc = dl % BLOCK
        buck = s // cfg.bucket_rows
        g = blk * nb + buck
        order = np.argsort(g, kind="stable")
        g_s, s_s, dloc_s = g[order], s[order], dloc[order]
        cnt = np.bincount(g_s, minlength=ngroups)
        cnt_all[m] = cnt
        sorted_per_core.append((s_s, dloc_s, np.concatenate([[0], np.cumsum(cnt)])))

    chunks_bk = -(-cnt_all.max(axis=0) // CHUNK)  # [ngroups] shared across cores
    chunks_bk = chunks_bk.reshape(nblk, nb)
    ck = chunks_bk.sum(axis=0)                    # chunks per bucket stream
    gk = -(-ck // GCH)                            # gather groups per bucket

    for m in range(NCORES):
        s_s, dloc_s, starts = sorted_per_core[m]
        streams = []
        for k in range(nb):
            tot = int(gk[k]) * GIDX
            idxs = np.zeros(tot, np.int64)
            dloc_arr = np.full(tot, 255, np.int64)
            pos = 0
            for b in range(nblk):
                gi = b * nb + k
                cnum = int(cnt_all[m, gi])
                st = int(starts[gi])
                idxs[pos:pos + cnum] = s_s[st:st + cnum] - k * cfg.bucket_rows
                dloc_arr[pos:pos + cnum] = dloc_s[st:st + cnum]
                pos += int(chunks_bk[b, k]) * CHUNK
            # wrapped int16 layout: index i of each 1024-group lives at
            # [i%16, i//16]; replicated over the 8 partition groups of 16.
            w16 = np.zeros((16, int(gk[k]) * 64), np.int16)
            for grp in range(int(gk[k])):
                fl = idxs[grp * GIDX:(grp + 1) * GIDX]
                w16[:, grp * 64:(grp + 1) * 64] = fl.reshape(64, 16).T
            idx_wrap = np.tile(w16, (8, 1))
            dl2 = dloc_arr.reshape(-1, CHUNK).T  # [128, chunks]
            streams.append((idx_wrap, dl2.astype(np.float32)))
        per_core_data.append(streams)

    return chunks_bk, gk, per_core_data, dinv


# ------------------------------------------------------------- program build

def build_program(cfg: Cfg, chunks_bk: np.ndarray, gk: np.ndarray):
    nblk, nb = cfg.nblk, cfg.nbuckets
    din, dh, dout, pc = cfg.din, cfg.dh, cfg.dout, cfg.per_core
    nkt = din // 128  # k-tiles for the x@W1 matmul

    nc = bacc.Bacc("TRN2", num_swdge_queues=nb)

    xT = nc.declare_dram_parameter("xT", [din, pc], f32, isOutput=False)
    w1 = nc.declare_dram_parameter("w1", [din, dh], f32, isOutput=False)
    w2 = nc.declare_dram_parameter("w2", [dh, dout], bf16, isOutput=False)
    b1r = nc.declare_dram_parameter("b1r", [128, dh], f32, isOutput=False)
    b2r = nc.declare_dram_parameter("b2r", [128, dout], f32, isOutput=False)
    dinv_in = nc.declare_dram_parameter("dinv", [128, nblk], f32, isOutput=False)
    idx_in = [
        nc.declare_dram_parameter(f"idx{k}", [128, int(gk[k]) * 64], i16, isOutput=False)
        for k in range(nb)
    ]
    dl_in = [
        nc.declare_dram_parameter(f"dl{k}", [128, int(gk[k]) * GCH], f32, isOutput=False)
        for k in range(nb)
    ]
    out2 = nc.declare_dram_parameter("out2", [pc, dout], f32, isOutput=True)

    bounce1 = nc.dram_tensor("bounce1", [pc, dh], f32)
    table1 = nc.dram_tensor("table1", [cfg.npad, dh], f32, addr_space="Shared")
    bounce2 = nc.dram_tensor("bounce2", [pc, dout], bf16)
    table2 = nc.dram_tensor("table2", [cfg.npad, dout], bf16, addr_space="Shared")
    rg = [list(range(NCORES))]

    with tile.TileContext(nc) as tc:
        with tc.tile_pool(name="const", bufs=1) as cpool:
            nc.gpsimd.load_library(mlp)

            # ---- constants
            w2_t = cpool.tile([dh, dout], bf16)
            nc.sync.dma_start(out=w2_t[:], in_=w2[:])
            b1_t = cpool.tile([128, dh], f32)
            nc.sync.dma_start(out=b1_t[:], in_=b1r[:])
            b2_t = cpool.tile([128, dout], f32)
            nc.sync.dma_start(out=b2_t[:], in_=b2r[:])
            dinv_t = cpool.tile([128, nblk], f32)
            nc.sync.dma_start(out=dinv_t[:], in_=dinv_in[:])
            iota_f = cpool.tile([128, 128], f32)
            nc.gpsimd.iota(iota_f[:], pattern=[[1, 128]], base=0, channel_multiplier=0,
                           allow_small_or_imprecise_dtypes=True)
            iota_b = cpool.tile([128, 128], bf16)
            nc.gpsimd.iota(iota_b[:], pattern=[[1, 128]], base=0, channel_multiplier=0,
                           allow_small_or_imprecise_dtypes=True)
            from concourse.masks import make_identity
            ident_f = cpool.tile([128, 128], f32)
            make_identity(nc, ident_f[:])
            ident_b = cpool.tile([128, 128], bf16)
            make_identity(nc, ident_b[:])

            h1s_own = cpool.tile([128, nblk * dh], f32, tag="h1s_own")
            z_own = cpool.tile([128, nblk * dout], bf16, tag="z_own")

            # ---- phase 1: h1s = (x @ W1) * dinv, keep own + send to bounce1
            with (
                tc.tile_pool(name="xtp", bufs=1) as xtp,
                tc.tile_pool(name="w1p", bufs=1) as w1p,
                tc.tile_pool(name="psh", bufs=2, space="PSUM") as pshp,
            ):
                xt_t = [xtp.tile([128, pc], f32, tag=f"xt{t}", name=f"xt{t}") for t in range(nkt)]
                for t in range(nkt):
                    nc.sync.dma_start(out=xt_t[t][:], in_=xT[t * 128:(t + 1) * 128, :])
                w1_t = [w1p.tile([128, dh], f32, tag=f"w1{t}", name=f"w1t{t}") for t in range(nkt)]
                for t in range(nkt):
                    nc.sync.dma_start(out=w1_t[t][:], in_=w1[t * 128:(t + 1) * 128, :])
                for b in range(nblk):
                    ph = pshp.tile([128, dh], f32, tag="psh")
                    for t in range(nkt):
                        nc.tensor.matmul(
                            ph[:], lhsT=xt_t[t][:, b * 128:(b + 1) * 128], rhs=w1_t[t][:],
                            start=(t == 0), stop=(t == nkt - 1),
                        )
                    nc.vector.tensor_scalar_mul(
                        h1s_own[:, b * dh:(b + 1) * dh], ph[:], dinv_t[:, b:b + 1])
                    nc.sync.dma_start(
                        out=bounce1[b * 128:(b + 1) * 128, :],
                        in_=h1s_own[:, b * dh:(b + 1) * dh])

            nc.gpsimd.collective_compute(
                "AllGather", mybir.AluOpType.bypass, replica_groups=rg,
                ins=[bounce1[:]], outs=[table1[:]])

            # ---- shared stream loads (both layers)
            idx_t = [cpool.tile([128, int(gk[k]) * 64], i16, tag=f"idx{k}", name=f"idxt{k}") for k in range(nb)]
            dl_f = [cpool.tile([128, int(gk[k]) * GCH], f32, tag=f"dlf{k}", name=f"dlf{k}") for k in range(nb)]
            dl_b = [cpool.tile([128, int(gk[k]) * GCH], bf16, tag=f"dlb{k}", name=f"dlb{k}") for k in range(nb)]
            for k in range(nb):
                nc.sync.dma_start(out=idx_t[k][:], in_=idx_in[k][:])
                nc.sync.dma_start(out=dl_f[k][:], in_=dl_in[k][:])
                nc.vector.tensor_copy(dl_b[k][:], dl_f[k][:])

            def aggregate(layer: int):
                """Emit gather + one-hot + matmul accumulation + eviction."""
                if layer == 1:
                    elem, table, ohdt, iota_t, ident, own = dh, table1, f32, iota_f, ident_f, h1s_own
                    dlv = dl_f
                else:
                    elem, table, ohdt, iota_t, ident, own = dout, table2, bf16, iota_b, ident_b, z_own
                    dlv = dl_b
                with (
                    tc.tile_pool(name=f"gat{layer}", bufs=3) as gpool,
                    tc.tile_pool(name=f"oh{layer}", bufs=3) as opool,
                    tc.tile_pool(name=f"ps{layer}", bufs=4, space="PSUM") as pspool,
                    tc.tile_pool(name=f"ev{layer}", bufs=3) as evpool,
                    tc.tile_pool(name=f"psz{layer}", bufs=2, space="PSUM") as pszpool,
                ):
                    nextg = [0] * nb
                    tiles = [None] * nb
                    qpos = [0] * nb
                    for b in range(nblk):
                        ps = pspool.tile([128, elem], f32, tag="ps")
                        nc.tensor.matmul(
                            ps[:], lhsT=ident[:],
                            rhs=own[:, b * elem:(b + 1) * elem],
                            start=True, stop=False)
                        nchunks = int(chunks_bk[b].sum())
                        done = 0
                        for k in range(nb):
                            for _ in range(int(chunks_bk[b, k])):
                                q = qpos[k]
                                grp, slot = q // GCH, q % GCH
                                if grp >= nextg[k]:
                                    gt = gpool.tile([128, GCH * elem], ohdt, tag=f"g{k}", name=f"gt{k}")
                                    ot = opool.tile([128, GCH * 128], ohdt, tag=f"o{k}", name=f"ot{k}")
                                    nc.gpsimd.dma_gather(
                                        gt[:].rearrange("p (c e) -> p c e", e=elem),
                                        table[k * cfg.bucket_rows:(k + 1) * cfg.bucket_rows, :],
                                        idx_t[k][:, grp * 64:(grp + 1) * 64],
                                        GIDX, GIDX, elem,
                                        single_packet=True, queue_num=k,
                                    )
                                    nc.vector.tensor_tensor(
                                        out=ot[:],
                                        in0=iota_t[:].rearrange("p (c f) -> p c f", c=1)
                                            .broadcast_to([128, GCH, 128]),
                                        in1=dlv[k][:, grp * GCH:(grp + 1) * GCH]
                                            .rearrange("p (c f) -> p c f", f=1)
                                            .broadcast_to([128, GCH, 128]),
                                        op=mybir.AluOpType.is_equal,
                                    )
                                    tiles[k] = (gt, ot)
                                    nextg[k] = grp + 1
                                gt, ot = tiles[k]
                                done += 1
                                nc.tensor.matmul(
                                    ps[:],
                                    lhsT=ot[:, slot * 128:(slot + 1) * 128],
                                    rhs=gt[:, slot * elem:(slot + 1) * elem],
                                    start=False, stop=(done == nchunks),
                                )
                                qpos[k] += 1
                        if layer == 1:
                            t1 = evpool.tile([128, dh], f32, tag="t1")
                            nc.vector.scalar_tensor_tensor(
                                out=t1[:], in0=ps[:], scalar=dinv_t[:, b:b + 1],
                                in1=b1_t[:], op0=mybir.AluOpType.mult,
                                op1=mybir.AluOpType.add)
                            r = evpool.tile([128, dh], f32, tag="r")
                            nc.scalar.activation(r[:], t1[:], mybir.ActivationFunctionType.Relu)
                            a1s = evpool.tile([128, dh], f32, tag="a1s")
                            nc.vector.tensor_scalar_mul(a1s[:], r[:], dinv_t[:, b:b + 1])
                            pT = pszpool.tile([dh, 128], f32, tag="pT")
                            nc.tensor.transpose(pT[:], a1s[:], ident_f[:])
                            a1sT = evpool.tile([dh, 128], bf16, tag="a1sT")
                            nc.scalar.activation(a1sT[:], pT[:], mybir.ActivationFunctionType.Copy)
                            pz = pszpool.tile([128, dout], f32, tag="pz")
                            nc.tensor.matmul(pz[:], lhsT=a1sT[:], rhs=w2_t[:],
                                             start=True, stop=True)
                            nc.scalar.activation(
                                z_own[:, b * dout:(b + 1) * dout], pz[:],
                                mybir.ActivationFunctionType.Copy)
                            nc.sync.dma_start(
                                out=bounce2[b * 128:(b + 1) * 128, :],
                                in_=z_own[:, b * dout:(b + 1) * dout])
                        else:
                            o = evpool.tile([128, dout], f32, tag="o")
                            nc.vector.scalar_tensor_tensor(
                                out=o[:], in0=ps[:], scalar=dinv_t[:, b:b + 1],
                                in1=b2_t[:], op0=mybir.AluOpType.mult,
                                op1=mybir.AluOpType.add)
                            nc.sync.dma_start(out=out2[b * 128:(b + 1) * 128, :], in_=o[:])

            aggregate(1)
            nc.gpsimd.collective_compute(
                "AllGather", mybir.AluOpType.bypass, replica_groups=rg,
                ins=[bounce2[:]], outs=[table2[:]])
            aggregate(2)

    nc.finalize()
    return nc


# ------------------------------------------------------------------- driver

def run(cfg: Cfg, x, edge_index, W1, b1, W2, b2, trace=False):
    x = np.asarray(x, np.float32)
    W1 = np.asarray(W1, np.float32)
    b1 = np.asarray(b1, np.float32)
    W2 = np.asarray(W2, np.float32)
    b2 = np.asarray(b2, np.float32)

    chunks_bk, gk, streams, dinv = host_prep(cfg, np.asarray(edge_index))
    nc = build_program(cfg, chunks_bk, gk)

    xpad = np.zeros((cfg.npad, cfg.din), np.float32)
    xpad[:cfg.n] = x
    b1_rep = np.tile(b1[None, :], (128, 1)).astype(np.float32)
    b2_rep = np.tile(b2[None, :], (128, 1)).astype(np.float32)
    w2b = W2.astype(np_bf16)

    in_maps = []
    for m in range(NCORES):
        sl = slice(m * cfg.per_core, (m + 1) * cfg.per_core)
        im = {
            "xT": np.ascontiguousarray(xpad[sl].T),
            "w1": W1,
            "w2": w2b,
            "b1r": b1_rep,
            "b2r": b2_rep,
            "dinv": np.ascontiguousarray(
                dinv[sl].reshape(cfg.nblk, 128).T),
        }
        for k in range(cfg.nbuckets):
            im[f"idx{k}"] = streams[m][k][0]
            im[f"dl{k}"] = streams[m][k][1]
        in_maps.append(im)

    res = run_bass_kernel_spmd(nc, in_maps, core_ids=list(range(NCORES)), trace=trace)
    out = np.concatenate([res.results[m]["out2"] for m in range(NCORES)], axis=0)
    return out[:cfg.n], res


def kernel(x, edge_index, W1, b1, W2, b2):
    out, _ = run(FULL_CFG, x, edge_index, W1, b1, W2, b2, trace=False)
    return out


# revision 4
# speedup vs baseline: 1.3389x; 1.3389x over previous
"""Trainium2 Bass kernel for a 2-layer GCN (nn_MiniGNN).

Math (per GCNConv, symmetric norm, self loops):
    deg[d] = indeg[d] + 1;  dinv = deg^-1/2
    out[d] = dinv[d] * ( sum_{e: dst=d} dinv[src_e] * h[src_e]  +  dinv[d]*h[d] ) + b
with h = x @ W.  Layer 1 applies relu;  layer 2 output = (S a1) @ W2 + b2,
computed as S @ (a1 @ W2) (associativity) so both layers aggregate via the
same machinery.

Implementation: 8-core SPMD.  Nodes (and their incoming edges) are sharded
by destination across cores.  Per core, edges are sorted by (dst-block of
128, src-bucket of 25088) and padded to 128-edge chunks.  Each chunk is
gathered (dma_gather, 1024 rows / instruction over 4 SWDGE queues) and
scatter-added into a PSUM accumulator via a one-hot matrix built on DVE
(is_equal against an iota row) and a PE matmul.  dinv factorizes into the
gathered table rows (src side) and the PSUM eviction scale (dst side);
self loops are identity-matmul'd from the core-local table slice; biases
are fused into eviction.  Between layers, table slices are exchanged with
an AllGather collective.
"""
import sys

sys.path.insert(0, "/opt/trn_rl_repo")

import numpy as np
import ml_dtypes
from dataclasses import dataclass

from concourse import bass, bacc, mybir
import concourse.tile as tile
from concourse.bass_utils import run_bass_kernel_spmd
from concourse.library_config import mlp

bf16 = mybir.dt.bfloat16
f32 = mybir.dt.float32
i16 = mybir.dt.int16
np_bf16 = ml_dtypes.bfloat16

NCORES = 8
BLOCK = 128          # dst nodes per psum block
CHUNK = 128          # edges per matmul chunk
GCH = 8              # chunks per dma_gather instruction
GIDX = CHUNK * GCH   # 1024 indices per gather


@dataclass
class Cfg:
    n: int           # real node count
    din: int
    dh: int
    dout: int
    bucket_rows: int  # src rows per int16 bucket (<= 32768)

    @property
    def nbuckets(self):
        return 4

    @property
    def npad(self):
        # multiple of NCORES*BLOCK and of nbuckets*... bucket_rows covers npad
        per = -(-self.n // (NCORES * BLOCK)) * BLOCK
        return per * NCORES

    @property
    def per_core(self):
        return self.npad // NCORES

    @property
    def nblk(self):
        return self.per_core // BLOCK


FULL_CFG = Cfg(n=100000, din=256, dh=64, dout=128, bucket_rows=25088)


# ----------------------------------------------------------------- host prep

def host_prep(cfg: Cfg, edge_index: np.ndarray):
    """Bucket/sort/pad edges per core. Returns (chunks_bk, per-core streams, dinv)."""
    src = np.asarray(edge_index[0], dtype=np.int64)
    dst = np.asarray(edge_index[1], dtype=np.int64)
    n, npad, per_core, nblk, nb = cfg.n, cfg.npad, cfg.per_core, cfg.nblk, cfg.nbuckets

    deg = np.bincount(dst, minlength=n).astype(np.float64) + 1.0
    dinv = (1.0 / np.sqrt(deg)).astype(np.float32)
    dinv = np.concatenate([dinv, np.ones(npad - n, np.float32)])

    core = dst // per_core
    ngroups = nblk * nb
    per_core_data = []
    cnt_all = np.zeros((NCORES, ngroups), np.int64)
    sorted_per_core = []
    for m in range(NCORES):
        sel = core == m
        s = src[sel]
        dl = dst[sel] - m * per_core
        blk = dl // BLOCK
        dloc = dl % BLOCK
        buck = s // cfg.bucket_rows
        g = blk * nb + buck
        order = np.argsort(g, kind="stable")
        g_s, s_s, dloc_s = g[order], s[order], dloc[order]
        cnt = np.bincount(g_s, minlength=ngroups)
        cnt_all[m] = cnt
        sorted_per_core.append((s_s, dloc_s, np.concatenate([[0], np.cumsum(cnt)])))

    chunks_bk = -(-cnt_all.max(axis=0) // CHUNK)  # [ngroups] shared across cores
    chunks_bk = chunks_bk.reshape(nblk, nb)
    ck = chunks_bk.sum(axis=0)                    # chunks per bucket stream
    gk = -(-ck // GCH)                            # gather groups per bucket

    for m in range(NCORES):
        s_s, dloc_s, starts = sorted_per_core[m]
        streams = []
        for k in range(nb):
            tot = int(gk[k]) * GIDX
            idxs = np.zeros(tot, np.int64)
            dloc_arr = np.full(tot, 255, np.int64)
            pos = 0
            for b in range(nblk):
                gi = b * nb + k
                cnum = int(cnt_all[m, gi])
                st = int(starts[gi])
                idxs[pos:pos + cnum] = s_s[st:st + cnum] - k * cfg.bucket_rows
                dloc_arr[pos:pos + cnum] = dloc_s[st:st + cnum]
                pos += int(chunks_bk[b, k]) * CHUNK
            # wrapped int16 layout: index i of each 1024-group lives at
            # [i%16, i//16]; replicated over the 8 partition groups of 16.
            w16 = np.zeros((16, int(gk[k]) * 64), np.int16)
            for grp in range(int(gk[k])):
                fl = idxs[grp * GIDX:(grp + 1) * GIDX]
                w16[:, grp * 64:(grp + 1) * 64] = fl.reshape(64, 16).T
            idx_wrap = np.tile(w16, (8, 1))
            dl2 = dloc_arr.reshape(-1, CHUNK).T  # [128, chunks]
            streams.append((idx_wrap, dl2.astype(np.float32)))
        per_core_data.append(streams)

    return chunks_bk, gk, per_core_data, dinv


# ------------------------------------------------------------- program build

def build_program(cfg: Cfg, chunks_bk: np.ndarray, gk: np.ndarray):
    nblk, nb = cfg.nblk, cfg.nbuckets
    din, dh, dout, pc = cfg.din, cfg.dh, cfg.dout, cfg.per_core
    nkt = din // 128  # k-tiles for the x@W1 matmul

    nc = bacc.Bacc("TRN2", num_swdge_queues=nb)

    xT = nc.declare_dram_parameter("xT", [din, pc], f32, isOutput=False)
    w1 = nc.declare_dram_parameter("w1", [din, dh], f32, isOutput=False)
    w2 = nc.declare_dram_parameter("w2", [dh, dout], bf16, isOutput=False)
    b1r = nc.declare_dram_parameter("b1r", [128, dh], f32, isOutput=False)
    b2r = nc.declare_dram_parameter("b2r", [128, dout], f32, isOutput=False)
    dinv_in = nc.declare_dram_parameter("dinv", [128, nblk], f32, isOutput=False)
    idx_in = [
        nc.declare_dram_parameter(f"idx{k}", [128, int(gk[k]) * 64], i16, isOutput=False)
        for k in range(nb)
    ]
    dl_in = [
        nc.declare_dram_parameter(f"dl{k}", [128, int(gk[k]) * GCH], bf16, isOutput=False)
        for k in range(nb)
    ]
    out2 = nc.declare_dram_parameter("out2", [pc, dout], f32, isOutput=True)

    bounce1 = nc.dram_tensor("bounce1", [pc, 2 * dh], bf16)
    table1 = nc.dram_tensor("table1", [cfg.npad, 2 * dh], bf16, addr_space="Shared")
    bounce2 = nc.dram_tensor("bounce2", [pc, dout], bf16)
    table2 = nc.dram_tensor("table2", [cfg.npad, dout], bf16, addr_space="Shared")
    rg = [list(range(NCORES))]

    with tile.TileContext(nc) as tc:
        with tc.tile_pool(name="const", bufs=1) as cpool:
            nc.gpsimd.load_library(mlp)

            # ---- constants
            w2_t = cpool.tile([dh, dout], bf16)
            nc.sync.dma_start(out=w2_t[:], in_=w2[:])
            b1_t = cpool.tile([128, dh], f32)
            nc.sync.dma_start(out=b1_t[:], in_=b1r[:])
            b2_t = cpool.tile([128, dout], f32)
            nc.sync.dma_start(out=b2_t[:], in_=b2r[:])
            dinv_t = cpool.tile([128, nblk], f32)
            nc.sync.dma_start(out=dinv_t[:], in_=dinv_in[:])
            iota_rep = cpool.tile([128, GCH * 128], bf16)
            nc.gpsimd.iota(iota_rep[:].rearrange("p (c f) -> p c f", c=GCH),
                           pattern=[[0, GCH], [1, 128]], base=0, channel_multiplier=0,
                           allow_small_or_imprecise_dtypes=True)
            from concourse.masks import make_identity
            ident_f = cpool.tile([128, 128], f32)
            make_identity(nc, ident_f[:])
            ident_b = cpool.tile([128, 128], bf16)
            make_identity(nc, ident_b[:])
            zero_t = cpool.tile([128, pc // 128 * dh], bf16)
            nc.vector.memset(zero_t[:], 0)
            nc.sync.dma_start(out=bounce1[:, dh:], in_=zero_t[:])

            h1s_own = cpool.tile([128, nblk * dh], bf16, tag="h1s_own")
            z_own = cpool.tile([128, nblk * dout], bf16, tag="z_own")

            # ---- phase 1: h1s = (x @ W1) * dinv, keep own + send to bounce1
            with (
                tc.tile_pool(name="xtp", bufs=1) as xtp,
                tc.tile_pool(name="w1p", bufs=1) as w1p,
                tc.tile_pool(name="psh", bufs=2, space="PSUM") as pshp,
            ):
                xt_t = [xtp.tile([128, pc], f32, tag=f"xt{t}", name=f"xt{t}") for t in range(nkt)]
                for t in range(nkt):
                    nc.sync.dma_start(out=xt_t[t][:], in_=xT[t * 128:(t + 1) * 128, :])
                w1_t = [w1p.tile([128, dh], f32, tag=f"w1{t}", name=f"w1t{t}") for t in range(nkt)]
                for t in range(nkt):
                    nc.sync.dma_start(out=w1_t[t][:], in_=w1[t * 128:(t + 1) * 128, :])
                for b in range(nblk):
                    ph = pshp.tile([128, dh], f32, tag="psh")
                    for t in range(nkt):
                        nc.tensor.matmul(
                            ph[:], lhsT=xt_t[t][:, b * 128:(b + 1) * 128], rhs=w1_t[t][:],
                            start=(t == 0), stop=(t == nkt - 1),
                        )
                    nc.scalar.activation(
                        h1s_own[:, b * dh:(b + 1) * dh], ph[:],
                        mybir.ActivationFunctionType.Copy, scale=dinv_t[:, b:b + 1])
                    nc.sync.dma_start(
                        out=bounce1[b * 128:(b + 1) * 128, :dh],
                        in_=h1s_own[:, b * dh:(b + 1) * dh])

            nc.gpsimd.collective_compute(
                "AllGather", mybir.AluOpType.bypass, replica_groups=rg,
                ins=[bounce1[:]], outs=[table1[:]])

            # ---- shared stream loads (both layers)
            idx_t = [cpool.tile([128, int(gk[k]) * 64], i16, tag=f"idx{k}", name=f"idxt{k}") for k in range(nb)]
            dl_b = [cpool.tile([128, int(gk[k]) * GCH], bf16, tag=f"dlb{k}", name=f"dlb{k}") for k in range(nb)]
            for k in range(nb):
                nc.sync.dma_start(out=idx_t[k][:], in_=idx_in[k][:])
                nc.sync.dma_start(out=dl_b[k][:], in_=dl_in[k][:])

            def aggregate(layer: int):
                """Emit gather + one-hot + matmul accumulation + eviction."""
                if layer == 1:
                    elem, feat, table, own = 2 * dh, dh, table1, h1s_own
                else:
                    elem, feat, table, own = dout, dout, table2, z_own
                ohdt, ident, dlv = bf16, ident_b, dl_b
                with (
                    tc.tile_pool(name=f"gat{layer}", bufs=3) as gpool,
                    tc.tile_pool(name=f"oh{layer}", bufs=3) as opool,
                    tc.tile_pool(name=f"ps{layer}", bufs=4, space="PSUM") as pspool,
                    tc.tile_pool(name=f"ev{layer}", bufs=3) as evpool,
                    tc.tile_pool(name=f"psz{layer}", bufs=2, space="PSUM") as pszpool,
                ):
                    nextg = [0] * nb
                    tiles = [None] * nb
                    qpos = [0] * nb
                    for b in range(nblk):
                        ps = pspool.tile([128, feat], f32, tag="ps")
                        nc.tensor.matmul(
                            ps[:], lhsT=ident[:],
                            rhs=own[:, b * feat:(b + 1) * feat],
                            start=True, stop=False)
                        nchunks = int(chunks_bk[b].sum())
                        done = 0
                        for k in range(nb):
                            for _ in range(int(chunks_bk[b, k])):
                                q = qpos[k]
                                grp, slot = q // GCH, q % GCH
                                if grp >= nextg[k]:
                                    gt = gpool.tile([128, GCH * elem], ohdt, tag=f"g{k}", name=f"gt{k}")
                                    ot = opool.tile([128, GCH * 128], ohdt, tag=f"o{k}", name=f"ot{k}")
                                    nc.gpsimd.dma_gather(
                                        gt[:].rearrange("p (c e) -> p c e", e=elem),
                                        table[k * cfg.bucket_rows:(k + 1) * cfg.bucket_rows, :],
                                        idx_t[k][:, grp * 64:(grp + 1) * 64],
                                        GIDX, GIDX, elem,
                                        single_packet=True, queue_num=k,
                                    )
                                    nc.vector.tensor_tensor(
                                        out=ot[:],
                                        in0=iota_rep[:],
                                        in1=dlv[k][:, grp * GCH:(grp + 1) * GCH]
                                            .rearrange("p (c f) -> p c f", f=1)
                                            .broadcast_to([128, GCH, 128]),
                                        op=mybir.AluOpType.is_equal,
                                    )
                                    tiles[k] = (gt, ot)
                                    nextg[k] = grp + 1
                                gt, ot = tiles[k]
                                done += 1
                                nc.tensor.matmul(
                                    ps[:],
                                    lhsT=ot[:, slot * 128:(slot + 1) * 128],
                                    rhs=gt[:, slot * elem:slot * elem + feat],
                                    start=False, stop=(done == nchunks),
                                )
                                qpos[k] += 1
                        if layer == 1:
                            t1 = evpool.tile([128, dh], f32, tag="t1")
                            nc.vector.scalar_tensor_tensor(
                                out=t1[:], in0=ps[:], scalar=dinv_t[:, b:b + 1],
                                in1=b1_t[:], op0=mybir.AluOpType.mult,
                                op1=mybir.AluOpType.add)
                            r = evpool.tile([128, dh], f32, tag="r")
                            nc.scalar.activation(r[:], t1[:], mybir.ActivationFunctionType.Relu)
                            a1s = evpool.tile([128, dh], f32, tag="a1s")
                            nc.scalar.activation(a1s[:], r[:], mybir.ActivationFunctionType.Copy,
                                                 scale=dinv_t[:, b:b + 1])
                            pT = pszpool.tile([dh, 128], f32, tag="pT")
                            nc.tensor.transpose(pT[:], a1s[:], ident_f[:])
                            a1sT = evpool.tile([dh, 128], bf16, tag="a1sT")
                            nc.scalar.activation(a1sT[:], pT[:], mybir.ActivationFunctionType.Copy)
                            pz = pszpool.tile([128, dout], f32, tag="pz")
                            nc.tensor.matmul(pz[:], lhsT=a1sT[:], rhs=w2_t[:],
                                             start=True, stop=True)
                            nc.scalar.activation(
                                z_own[:, b * dout:(b + 1) * dout], pz[:],
                                mybir.ActivationFunctionType.Copy)
                            nc.sync.dma_start(
                                out=bounce2[b * 128:(b + 1) * 128, :],
                                in_=z_own[:, b * dout:(b + 1) * dout])
                        else:
                            o = evpool.tile([128, dout], f32, tag="o")
                            nc.vector.scalar_tensor_tensor(
                                out=o[:], in0=ps[:], scalar=dinv_t[:, b:b + 1],
                                in1=b2_t[:], op0=mybir.AluOpType.mult,
                                op1=mybir.AluOpType.add)
                            nc.sync.dma_start(out=out2[b * 128:(b + 1) * 128, :], in_=o[:])

            aggregate(1)
            nc.gpsimd.collective_compute(
                "AllGather", mybir.AluOpType.bypass, replica_groups=rg,
                ins=[bounce2[:]], outs=[table2[:]])
            aggregate(2)

    nc.finalize()
    return nc


# ------------------------------------------------------------------- driver

def run(cfg: Cfg, x, edge_index, W1, b1, W2, b2, trace=False):
    x = np.asarray(x, np.float32)
    W1 = np.asarray(W1, np.float32)
    b1 = np.asarray(b1, np.float32)
    W2 = np.asarray(W2, np.float32)
    b2 = np.asarray(b2, np.float32)

    chunks_bk, gk, streams, dinv = host_prep(cfg, np.asarray(edge_index))
    nc = build_program(cfg, chunks_bk, gk)

    xpad = np.zeros((cfg.npad, cfg.din), np.float32)
    xpad[:cfg.n] = x
    b1_rep = np.tile(b1[None, :], (128, 1)).astype(np.float32)
    b2_rep = np.tile(b2[None, :], (128, 1)).astype(np.float32)
    w2b = W2.astype(np_bf16)

    in_maps = []
    for m in range(NCORES):
        sl = slice(m * cfg.per_core, (m + 1) * cfg.per_core)
        im = {
            "xT": np.ascontiguousarray(xpad[sl].T),
            "w1": W1,
            "w2": w2b,
            "b1r": b1_rep,
            "b2r": b2_rep,
            "dinv": np.ascontiguousarray(
                dinv[sl].reshape(cfg.nblk, 128).T),
        }
        for k in range(cfg.nbuckets):
            im[f"idx{k}"] = streams[m][k][0]
            im[f"dl{k}"] = streams[m][k][1].astype(np_bf16)
        in_maps.append(im)

    res = run_bass_kernel_spmd(nc, in_maps, core_ids=list(range(NCORES)), trace=trace)
    out = np.concatenate([res.results[m]["out2"] for m in range(NCORES)], axis=0)
    return out[:cfg.n], res


def kernel(x, edge_index, W1, b1, W2, b2):
    out, _ = run(FULL_CFG, x, edge_index, W1, b1, W2, b2, trace=False)
    return out


# revision 5
# speedup vs baseline: 1.4072x; 1.0510x over previous
"""Trainium2 Bass kernel for a 2-layer GCN (nn_MiniGNN).

Math (per GCNConv, symmetric norm, self loops):
    deg[d] = indeg[d] + 1;  dinv = deg^-1/2
    out[d] = dinv[d] * ( sum_{e: dst=d} dinv[src_e] * h[src_e]  +  dinv[d]*h[d] ) + b
with h = x @ W.  Layer 1 applies relu;  layer 2 output = (S a1) @ W2 + b2,
computed as S @ (a1 @ W2) (associativity) so both layers aggregate via the
same machinery.

Implementation: 8-core SPMD.  Nodes (and their incoming edges) are sharded
by destination across cores.  Per core, edges are sorted by (dst-block of
128, src-bucket of 25088) and padded to 128-edge chunks.  Each chunk is
gathered (dma_gather, 1024 rows / instruction over 4 SWDGE queues) and
scatter-added into a PSUM accumulator via a one-hot matrix built on DVE
(is_equal against an iota row) and a PE matmul.  dinv factorizes into the
gathered table rows (src side) and the PSUM eviction scale (dst side);
self loops are identity-matmul'd from the core-local table slice; biases
are fused into eviction.  Between layers, table slices are exchanged with
an AllGather collective.
"""
import sys

sys.path.insert(0, "/opt/trn_rl_repo")

import numpy as np
import ml_dtypes
from dataclasses import dataclass

from concourse import bass, bacc, mybir
import concourse.tile as tile
from concourse.bass_utils import run_bass_kernel_spmd
from concourse.library_config import mlp

bf16 = mybir.dt.bfloat16
f32 = mybir.dt.float32
i16 = mybir.dt.int16
np_bf16 = ml_dtypes.bfloat16

NCORES = 8
BLOCK = 128          # dst nodes per psum block
CHUNK = 128          # edges per matmul chunk
GCH = 8              # chunks per dma_gather instruction
GIDX = CHUNK * GCH   # 1024 indices per gather


@dataclass
class Cfg:
    n: int           # real node count
    din: int
    dh: int
    dout: int

    @property
    def nbuckets(self):
        return 4

    @property
    def chunk_blocks(self):
        # dst-blocks per allgather chunk (= per gather bucket)
        q, r = divmod(self.nblk, self.nbuckets)
        return [q + 1] * r + [q] * (self.nbuckets - r)

    @property
    def chunk_rows(self):
        return [b * BLOCK for b in self.chunk_blocks]

    @property
    def chunk_starts(self):
        out = [0]
        for r in self.chunk_rows:
            out.append(out[-1] + r)
        return out  # length nbuckets+1, last == per_core

    @property
    def bucket_rows_k(self):
        return [NCORES * r for r in self.chunk_rows]

    @property
    def bucket_base(self):
        return [NCORES * s for s in self.chunk_starts[:-1]]

    @property
    def npad(self):
        # multiple of NCORES*BLOCK and of nbuckets*... bucket_rows covers npad
        per = -(-self.n // (NCORES * BLOCK)) * BLOCK
        return per * NCORES

    @property
    def per_core(self):
        return self.npad // NCORES

    @property
    def nblk(self):
        return self.per_core // BLOCK


FULL_CFG = Cfg(n=100000, din=256, dh=64, dout=128)


# ----------------------------------------------------------------- host prep

def host_prep(cfg: Cfg, edge_index: np.ndarray):
    """Bucket/sort/pad edges per core. Returns (chunks_bk, per-core streams, dinv)."""
    src = np.asarray(edge_index[0], dtype=np.int64)
    dst = np.asarray(edge_index[1], dtype=np.int64)
    n, npad, per_core, nblk, nb = cfg.n, cfg.npad, cfg.per_core, cfg.nblk, cfg.nbuckets

    deg = np.bincount(dst, minlength=n).astype(np.float64) + 1.0
    dinv = (1.0 / np.sqrt(deg)).astype(np.float32)
    dinv = np.concatenate([dinv, np.ones(npad - n, np.float32)])

    core = dst // per_core
    ngroups = nblk * nb
    per_core_data = []
    cnt_all = np.zeros((NCORES, ngroups), np.int64)
    sorted_per_core = []
    cstarts = np.asarray(cfg.chunk_starts, np.int64)
    crows = np.asarray(cfg.chunk_rows, np.int64)
    for m in range(NCORES):
        sel = core == m
        s = src[sel]
        dl = dst[sel] - m * per_core
        blk = dl // BLOCK
        dloc = dl % BLOCK
        s_r = s // per_core
        s_l = s % per_core
        buck = np.searchsorted(cstarts, s_l, side="right") - 1
        inb = s_r * crows[buck] + (s_l - cstarts[buck])
        g = blk * nb + buck
        order = np.argsort(g, kind="stable")
        g_s, s_s, dloc_s = g[order], inb[order], dloc[order]
        cnt = np.bincount(g_s, minlength=ngroups)
        cnt_all[m] = cnt
        sorted_per_core.append((s_s, dloc_s, np.concatenate([[0], np.cumsum(cnt)])))

    chunks_bk = -(-cnt_all.max(axis=0) // CHUNK)  # [ngroups] shared across cores
    chunks_bk = chunks_bk.reshape(nblk, nb)
    ck = chunks_bk.sum(axis=0)                    # chunks per bucket stream
    gk = -(-ck // GCH)                            # gather groups per bucket

    for m in range(NCORES):
        s_s, dloc_s, starts = sorted_per_core[m]
        streams = []
        for k in range(nb):
            tot = int(gk[k]) * GIDX
            idxs = np.zeros(tot, np.int64)
            dloc_arr = np.full(tot, 255, np.int64)
            pos = 0
            for b in range(nblk):
                gi = b * nb + k
                cnum = int(cnt_all[m, gi])
                st = int(starts[gi])
                idxs[pos:pos + cnum] = s_s[st:st + cnum]
                dloc_arr[pos:pos + cnum] = dloc_s[st:st + cnum]
                pos += int(chunks_bk[b, k]) * CHUNK
            # wrapped int16 layout: index i of each 1024-group lives at
            # [i%16, i//16]; replicated over the 8 partition groups of 16.
            w16 = np.zeros((16, int(gk[k]) * 64), np.int16)
            for grp in range(int(gk[k])):
                fl = idxs[grp * GIDX:(grp + 1) * GIDX]
                w16[:, grp * 64:(grp + 1) * 64] = fl.reshape(64, 16).T
            idx_wrap = np.tile(w16, (8, 1))
            dl2 = dloc_arr.reshape(-1, CHUNK).T  # [128, chunks]
            streams.append((idx_wrap, dl2.astype(np.float32)))
        per_core_data.append(streams)

    return chunks_bk, gk, per_core_data, dinv


# ------------------------------------------------------------- program build

def build_program(cfg: Cfg, chunks_bk: np.ndarray, gk: np.ndarray):
    nblk, nb = cfg.nblk, cfg.nbuckets
    din, dh, dout, pc = cfg.din, cfg.dh, cfg.dout, cfg.per_core
    nkt = din // 128  # k-tiles for the x@W1 matmul

    nc = bacc.Bacc("TRN2", num_swdge_queues=nb)

    xT = nc.declare_dram_parameter("xT", [din, pc], bf16, isOutput=False)
    w1 = nc.declare_dram_parameter("w1", [din, dh], bf16, isOutput=False)
    w2 = nc.declare_dram_parameter("w2", [dh, dout], bf16, isOutput=False)
    b1r = nc.declare_dram_parameter("b1r", [128, dh], f32, isOutput=False)
    b2r = nc.declare_dram_parameter("b2r", [128, dout], f32, isOutput=False)
    dinv_in = nc.declare_dram_parameter("dinv", [128, nblk], f32, isOutput=False)
    idx_in = [
        nc.declare_dram_parameter(f"idx{k}", [128, int(gk[k]) * 64], i16, isOutput=False)
        for k in range(nb)
    ]
    dl_in = [
        nc.declare_dram_parameter(f"dl{k}", [128, int(gk[k]) * GCH], bf16, isOutput=False)
        for k in range(nb)
    ]
    out2 = nc.declare_dram_parameter("out2", [pc, dout], f32, isOutput=True)

    cst = cfg.chunk_starts
    bb = cfg.bucket_base
    brk = cfg.bucket_rows_k
    cb = cfg.chunk_blocks
    bnd_blocks = []
    acc = 0
    for x_ in cb:
        acc += x_
        bnd_blocks.append(acc)

    bounce1 = nc.dram_tensor("bounce1", [pc, 2 * dh], bf16)
    table1 = nc.dram_tensor("table1", [cfg.npad, 2 * dh], bf16, addr_space="Shared")
    bounce2 = nc.dram_tensor("bounce2", [pc, dout], bf16)
    table2 = nc.dram_tensor("table2", [cfg.npad, dout], bf16, addr_space="Shared")
    rg = [list(range(NCORES))]

    with tile.TileContext(nc) as tc:
        with tc.tile_pool(name="const", bufs=1) as cpool:
            nc.gpsimd.load_library(mlp)

            # ---- constants
            w2_t = cpool.tile([dh, dout], bf16)
            nc.sync.dma_start(out=w2_t[:], in_=w2[:])
            b1_t = cpool.tile([128, dh], f32)
            nc.sync.dma_start(out=b1_t[:], in_=b1r[:])
            b2_t = cpool.tile([128, dout], f32)
            nc.sync.dma_start(out=b2_t[:], in_=b2r[:])
            dinv_t = cpool.tile([128, nblk], f32)
            nc.sync.dma_start(out=dinv_t[:], in_=dinv_in[:])
            iota_rep = cpool.tile([128, GCH * 128], bf16)
            nc.gpsimd.iota(iota_rep[:].rearrange("p (c f) -> p c f", c=GCH),
                           pattern=[[0, GCH], [1, 128]], base=0, channel_multiplier=0,
                           allow_small_or_imprecise_dtypes=True)
            from concourse.masks import make_identity
            ident_f = cpool.tile([128, 128], f32)
            make_identity(nc, ident_f[:])
            ident_b = cpool.tile([128, 128], bf16)
            make_identity(nc, ident_b[:])
            zero_t = cpool.tile([128, pc // 128 * dh], bf16)
            nc.vector.memset(zero_t[:], 0)
            nc.sync.dma_start(out=bounce1[:, dh:], in_=zero_t[:])

            h1s_own = cpool.tile([128, nblk * dh], bf16, tag="h1s_own")
            z_own = cpool.tile([128, nblk * dout], bf16, tag="z_own")

            # ---- phase 1: h1s = (x @ W1) * dinv, keep own + send to bounce1
            with (
                tc.tile_pool(name="xtp", bufs=1) as xtp,
                tc.tile_pool(name="w1p", bufs=1) as w1p,
                tc.tile_pool(name="psh", bufs=2, space="PSUM") as pshp,
            ):
                xt_t = [xtp.tile([128, pc], bf16, tag=f"xt{t}", name=f"xt{t}") for t in range(nkt)]
                for t in range(nkt):
                    nc.sync.dma_start(out=xt_t[t][:], in_=xT[t * 128:(t + 1) * 128, :])
                w1_t = [w1p.tile([128, dh], bf16, tag=f"w1{t}", name=f"w1t{t}") for t in range(nkt)]
                for t in range(nkt):
                    nc.sync.dma_start(out=w1_t[t][:], in_=w1[t * 128:(t + 1) * 128, :])
                for b in range(nblk):
                    ph = pshp.tile([128, dh], f32, tag="psh")
                    for t in range(nkt):
                        nc.tensor.matmul(
                            ph[:], lhsT=xt_t[t][:, b * 128:(b + 1) * 128], rhs=w1_t[t][:],
                            start=(t == 0), stop=(t == nkt - 1),
                        )
                    nc.scalar.activation(
                        h1s_own[:, b * dh:(b + 1) * dh], ph[:],
                        mybir.ActivationFunctionType.Copy, scale=dinv_t[:, b:b + 1])
                    nc.sync.dma_start(
                        out=bounce1[b * 128:(b + 1) * 128, :dh],
                        in_=h1s_own[:, b * dh:(b + 1) * dh])
                    if b + 1 in bnd_blocks:
                        kc = bnd_blocks.index(b + 1)
                        nc.gpsimd.collective_compute(
                            "AllGather", mybir.AluOpType.bypass, replica_groups=rg,
                            ins=[bounce1[cst[kc]:cst[kc + 1], :]],
                            outs=[table1[bb[kc]:bb[kc] + brk[kc], :]])

            # ---- shared stream loads (both layers)
            idx_t = [cpool.tile([128, int(gk[k]) * 64], i16, tag=f"idx{k}", name=f"idxt{k}") for k in range(nb)]
            dl_b = [cpool.tile([128, int(gk[k]) * GCH], bf16, tag=f"dlb{k}", name=f"dlb{k}") for k in range(nb)]
            for k in range(nb):
                nc.sync.dma_start(out=idx_t[k][:], in_=idx_in[k][:])
                nc.sync.dma_start(out=dl_b[k][:], in_=dl_in[k][:])

            def aggregate(layer: int):
                """Emit gather + one-hot + matmul accumulation + eviction."""
                if layer == 1:
                    elem, feat, table, own = 2 * dh, dh, table1, h1s_own
                else:
                    elem, feat, table, own = dout, dout, table2, z_own
                ohdt, ident, dlv = bf16, ident_b, dl_b
                with (
                    tc.tile_pool(name=f"gat{layer}", bufs=3) as gpool,
                    tc.tile_pool(name=f"oh{layer}", bufs=3) as opool,
                    tc.tile_pool(name=f"ps{layer}", bufs=4, space="PSUM") as pspool,
                    tc.tile_pool(name=f"ev{layer}", bufs=3) as evpool,
                    tc.tile_pool(name=f"psz{layer}", bufs=2, space="PSUM") as pszpool,
                ):
                    nextg = [0] * nb
                    tiles = [None] * nb
                    qpos = [0] * nb
                    for b in range(nblk):
                        ps = pspool.tile([128, feat], f32, tag="ps")
                        nc.tensor.matmul(
                            ps[:], lhsT=ident[:],
                            rhs=own[:, b * feat:(b + 1) * feat],
                            start=True, stop=False)
                        nchunks = int(chunks_bk[b].sum())
                        done = 0
                        for k in range(nb):
                            for _ in range(int(chunks_bk[b, k])):
                                q = qpos[k]
                                grp, slot = q // GCH, q % GCH
                                if grp >= nextg[k]:
                                    gt = gpool.tile([128, GCH * elem], ohdt, tag=f"g{k}", name=f"gt{k}")
                                    ot = opool.tile([128, GCH * 128], ohdt, tag=f"o{k}", name=f"ot{k}")
                                    nc.gpsimd.dma_gather(
                                        gt[:].rearrange("p (c e) -> p c e", e=elem),
                                        table[bb[k]:bb[k] + brk[k], :],
                                        idx_t[k][:, grp * 64:(grp + 1) * 64],
                                        GIDX, GIDX, elem,
                                        single_packet=True, queue_num=k,
                                    )
                                    nc.vector.tensor_tensor(
                                        out=ot[:],
                                        in0=iota_rep[:],
                                        in1=dlv[k][:, grp * GCH:(grp + 1) * GCH]
                                            .rearrange("p (c f) -> p c f", f=1)
                                            .broadcast_to([128, GCH, 128]),
                                        op=mybir.AluOpType.is_equal,
                                    )
                                    tiles[k] = (gt, ot)
                                    nextg[k] = grp + 1
                                gt, ot = tiles[k]
                                done += 1
                                nc.tensor.matmul(
                                    ps[:],
                                    lhsT=ot[:, slot * 128:(slot + 1) * 128],
                                    rhs=gt[:, slot * elem:slot * elem + feat],
                                    start=False, stop=(done == nchunks),
                                )
                                qpos[k] += 1
                        if layer == 1:
                            t1 = evpool.tile([128, dh], f32, tag="t1")
                            nc.vector.scalar_tensor_tensor(
                                out=t1[:], in0=ps[:], scalar=dinv_t[:, b:b + 1],
                                in1=b1_t[:], op0=mybir.AluOpType.mult,
                                op1=mybir.AluOpType.add)
                            r = evpool.tile([128, dh], f32, tag="r")
                            nc.scalar.activation(r[:], t1[:], mybir.ActivationFunctionType.Relu)
                            a1s = evpool.tile([128, dh], f32, tag="a1s")
                            nc.scalar.activation(a1s[:], r[:], mybir.ActivationFunctionType.Copy,
                                                 scale=dinv_t[:, b:b + 1])
                            pT = pszpool.tile([dh, 128], f32, tag="pT")
                            nc.tensor.transpose(pT[:], a1s[:], ident_f[:])
                            a1sT = evpool.tile([dh, 128], bf16, tag="a1sT")
                            nc.scalar.activation(a1sT[:], pT[:], mybir.ActivationFunctionType.Copy)
                            pz = pszpool.tile([128, dout], f32, tag="pz")
                            nc.tensor.matmul(pz[:], lhsT=a1sT[:], rhs=w2_t[:],
                                             start=True, stop=True)
                            nc.scalar.activation(
                                z_own[:, b * dout:(b + 1) * dout], pz[:],
                                mybir.ActivationFunctionType.Copy)
                            nc.sync.dma_start(
                                out=bounce2[b * 128:(b + 1) * 128, :],
                                in_=z_own[:, b * dout:(b + 1) * dout])
                            if b + 1 in bnd_blocks:
                                kc = bnd_blocks.index(b + 1)
                                nc.gpsimd.collective_compute(
                                    "AllGather", mybir.AluOpType.bypass,
                                    replica_groups=rg,
                                    ins=[bounce2[cst[kc]:cst[kc + 1], :]],
                                    outs=[table2[bb[kc]:bb[kc] + brk[kc], :]])
                        else:
                            o = evpool.tile([128, dout], f32, tag="o")
                            nc.vector.scalar_tensor_tensor(
                                out=o[:], in0=ps[:], scalar=dinv_t[:, b:b + 1],
                                in1=b2_t[:], op0=mybir.AluOpType.mult,
                                op1=mybir.AluOpType.add)
                            nc.sync.dma_start(out=out2[b * 128:(b + 1) * 128, :], in_=o[:])

            aggregate(1)
            aggregate(2)

    nc.finalize()
    return nc


# ------------------------------------------------------------------- driver

def run(cfg: Cfg, x, edge_index, W1, b1, W2, b2, trace=False):
    x = np.asarray(x, np.float32)
    W1 = np.asarray(W1, np.float32)
    b1 = np.asarray(b1, np.float32)
    W2 = np.asarray(W2, np.float32)
    b2 = np.asarray(b2, np.float32)

    chunks_bk, gk, streams, dinv = host_prep(cfg, np.asarray(edge_index))
    nc = build_program(cfg, chunks_bk, gk)

    xpad = np.zeros((cfg.npad, cfg.din), np.float32)
    xpad[:cfg.n] = x
    b1_rep = np.tile(b1[None, :], (128, 1)).astype(np.float32)
    b2_rep = np.tile(b2[None, :], (128, 1)).astype(np.float32)
    w2b = W2.astype(np_bf16)

    in_maps = []
    for m in range(NCORES):
        sl = slice(m * cfg.per_core, (m + 1) * cfg.per_core)
        im = {
            "xT": np.ascontiguousarray(xpad[sl].T).astype(np_bf16),
            "w1": W1.astype(np_bf16),
            "w2": w2b,
            "b1r": b1_rep,
            "b2r": b2_rep,
            "dinv": np.ascontiguousarray(
                dinv[sl].reshape(cfg.nblk, 128).T),
        }
        for k in range(cfg.nbuckets):
            im[f"idx{k}"] = streams[m][k][0]
            im[f"dl{k}"] = streams[m][k][1].astype(np_bf16)
        in_maps.append(im)

    res = run_bass_kernel_spmd(nc, in_maps, core_ids=list(range(NCORES)), trace=trace)
    out = np.concatenate([res.results[m]["out2"] for m in range(NCORES)], axis=0)
    return out[:cfg.n], res


def kernel(x, edge_index, W1, b1, W2, b2):
    out, _ = run(FULL_CFG, x, edge_index, W1, b1, W2, b2, trace=False)
    return out
